# revision 15
# baseline (speedup 1.0000x reference)
"""MoE grouped linear (gmm) kernel for 8 Trainium2 NeuronCores.

Strategy (load-balanced expert parallel):
  - Tokens arrive pre-sorted by expert. Instead of one expert per core
    (which pads every core to the largest group), each core processes two
    fixed-size token slots (S0, S1): a bin-packing DP assigns each expert
    a_e slots of size S0 and b_e of size S1 (8 of each across the chip)
    so that a_e*S0 + b_e*S1 >= g_e. For the reference group sizes this
    yields (320, 224) = 544 tokens/core vs 768 for plain expert parallel.
  - All tensors are converted to bf16 host-side (round-to-nearest), which
    halves HBM traffic vs shipping fp32 and truncating on-chip; the output
    is returned as bf16 and upcast host-side (tolerance is 2e-2, measured
    error ~4e-3).
  - Per core: y^T[o-block] += W[job][o-block,k]^T @ X^T[k, slot] over 16
    k-blocks, accumulating in PSUM (one bank per (o%4, job)); evacuation
    fuses the bias add on the vector engine and writes bf16.
  - DMA: W slabs stream on the sync queue (per-o fused job pairs, 8 KiB
    per partition); X streams on the scalar queue so the two queues issue
    descriptors in parallel (each DMA_DIRECT2D costs ~0.7 us of issuing
    engine time). The o=0 job-A weights and the first X quarter are split
    fine so the first real matmul starts as early as possible after the
    ~7.6 us runtime init; HAM warmup matmuls bridge the wait.
Host then scatters per-slot outputs back to [T, Out] fp32.
"""

import numpy as np
import ml_dtypes

import concourse.bass as bass
from concourse import bacc
import concourse.mybir as mybir
import concourse.tile as tile
from concourse.bass_utils import run_bass_kernel_spmd

N_CORES = 8
P = 128
NSLOT = 8  # slots per size class (one per core)

_BUILD_CACHE: dict = {}


def _pack(g: list[int]):
    """Choose slot sizes (S0 >= S1) and per-expert slot counts.

    Returns (S0, S1, assign) with assign[e] = (a_e, b_e): expert e's tokens
    are covered by a_e slots of S0 tokens plus b_e slots of S1 tokens,
    sum(a) <= 8, sum(b) <= 8. Minimizes S0+S1 (per-core PE work), then the
    LDWEIGHTS-aware stream time max(S0,256)+max(S1,256).
    """
    E = len(g)
    total = sum(g)
    tc_min = max(64, -(-total // N_CORES // 32) * 32)

    def feasible(S0, S1):
        states = {(0, 0): None}
        hist = []
        for e in range(E):
            opts = []
            for a in range(NSLOT + 1):
                need = g[e] - a * S0
                b = max(0, -(-need // S1))
                if b <= NSLOT:
                    opts.append((a, b))
            new = {}
            for (A, B) in states:
                for (a, b) in opts:
                    if A + a <= NSLOT and B + b <= NSLOT and (A + a, B + b) not in new:
                        new[(A + a, B + b)] = (A, B, a, b)
            hist.append(new)
            states = new
            if not states:
                return None
        key = next(iter(states))
        assign = [None] * E
        for e in range(E - 1, -1, -1):
            A, B, a, b = hist[e][key]
            assign[e] = (a, b)
            key = (A, B)
        return assign

    for TC in range(tc_min, 1025, 32):
        cands = []
        for S1 in range(32, TC // 2 + 1, 32):
            S0 = TC - S1
            if S0 > 512:
                continue
            cands.append((max(S0, 256) + max(S1, 256), -S1, S0, S1))
        for _, _, S0, S1 in sorted(cands):
            asg = feasible(S0, S1)
            if asg is not None:
                return S0, S1, asg
    raise ValueError(f"no 2-slot packing found for group sizes {g}")


def _slots_from_assign(g, S0, S1, assign):
    """Materialize assign into 16 slots: [(expert, start_within_group, len)].
    Slots 0..7 have capacity S0 (one per core), 8..15 capacity S1."""
    s0_slots, s1_slots = [], []
    for e, (a, b) in enumerate(assign):
        rem, pos = g[e], 0
        for _ in range(a):
            take = min(S0, rem)
            s0_slots.append((e, pos, take))
            pos += take
            rem -= take
        for _ in range(b):
            take = min(S1, rem)
            s1_slots.append((e, pos, take))
            pos += take
            rem -= take
        assert rem == 0
    s0_slots += [(0, 0, 0)] * (NSLOT - len(s0_slots))
    s1_slots += [(0, 0, 0)] * (NSLOT - len(s1_slots))
    return s0_slots + s1_slots


def _build_program(S0: int, S1: int, n_in: int, n_out: int):
    kb = n_in // P   # contraction blocks
    ob = n_out // P  # output-row blocks
    TC = S0 + S1
    f32 = mybir.dt.float32
    bf16 = mybir.dt.bfloat16

    nc = bacc.Bacc(
        "TRN2", target_bir_lowering=False, debug=False, num_devices=N_CORES
    )
    # X^T resident in SBUF: [P(partition of k-block), kb, TC] bf16.
    xt = nc.dram_tensor("xt", [P, kb, TC], bf16, kind="ExternalInput")
    # W pre-tiled host-side: [ob, P(k within block), job, kb, P(o)] so one
    # fused per-o DMA moves both jobs' slabs as 8 KiB/partition segments.
    w = nc.dram_tensor("w", [ob, P, 2, kb, P], bf16, kind="ExternalInput")
    bias = nc.dram_tensor("bias", [P, ob], f32, kind="ExternalInput")
    yt = nc.dram_tensor("yt", [n_out, TC], bf16, kind="ExternalOutput")

    KLO = 4          # o=0 weight split: k < KLO arrives first

    with tile.TileContext(nc) as tc:
        with (
            tc.tile_pool(name="const", bufs=1) as constp,
            tc.tile_pool(name="xsb", bufs=1) as xp,
            tc.tile_pool(name="w0sb", bufs=1) as w0p,
            tc.tile_pool(name="wsb", bufs=8) as wp,
            tc.tile_pool(name="osb", bufs=4) as outp,
            tc.tile_pool(name="psum", bufs=1, space="PSUM") as psump,
        ):
            bias_sb = constp.tile([P, ob], f32)
            nc.scalar.dma_start(bias_sb[:], bias[:])

            def load_x(tag, k0, nk, eng):
                t = xp.tile([P, nk, TC], bf16, tag=tag, name=tag)
                eng.dma_start(t[:], xt[:, k0 : k0 + nk, :])
                return t

            def load_w0(j, k0, k1, tag, eng):
                t = w0p.tile([P, k1 - k0, P], bf16, tag=tag, name=tag)
                eng.dma_start(t[:], w[0][:, j, k0:k1, :])
                return t

            # Balanced dual-queue load split: both HWDGE queues carry the
            # load stream, alternating pieces in matched priority order so
            # neither queue ever holds bytes more urgent than the other
            # (an unbalanced class split starves the weight stream behind
            # the round-robin). Two queues double the outstanding-DMA
            # pipeline depth, which is what bounds effective HBM rate.
            w0a_lo = load_w0(0, 0, KLO, "w0alo", nc.sync)
            w0b_lo = load_w0(1, 0, KLO, "w0blo", nc.scalar)
            xq0a = load_x("x0a", 0, 2, nc.sync)
            xq0b = load_x("x0b", 2, 2, nc.scalar)
            w0a_hi = load_w0(0, KLO, kb, "w0ahi", nc.sync)
            w0b_hi = load_w0(1, KLO, kb, "w0bhi", nc.scalar)
            xqs = [
                load_x("x1", 4, 4, nc.sync),
                load_x("x2", 8, 4, nc.scalar),
                load_x("x3", 12, 4, nc.sync),
            ]

            # o=1's weights also arrive as lo/hi pieces: the single ~3.5 us
            # wait for a full w1 slab after the X tail re-throttles HAM (any
            # PE-idle gap > ~3.4 us drops the clock to 1.2 GHz); several
            # sub-window waits keep the PE clock at 2.4 GHz into the steady
            # stream.
            def load_w1(j, k0, k1, tag, eng):
                t = w0p.tile([P, k1 - k0, P], bf16, tag=tag, name=tag)
                eng.dma_start(t[:], w[1][:, j, k0:k1, :])
                return t

            w1 = {
                (0, 0): load_w1(0, 0, KLO, "w1alo", nc.sync),
                (0, 1): load_w1(0, KLO, kb, "w1ahi", nc.scalar),
                (1, 0): load_w1(1, 0, KLO, "w1blo", nc.sync),
                (1, 1): load_w1(1, KLO, kb, "w1bhi", nc.scalar),
            }

            def w1k(j, k):
                if k < KLO:
                    return w1[(j, 0)][:, k, :]
                return w1[(j, 1)][:, k - KLO, :]

            def xk(k, t0, S):
                if k < 2:
                    return xq0a[:, k, t0 : t0 + S]
                if k < 4:
                    return xq0b[:, k - 2, t0 : t0 + S]
                q = k // 4
                return xqs[q - 1][:, k % 4, t0 : t0 + S]

            def w0k(j, k):
                lo, hi = (w0a_lo, w0a_hi) if j == 0 else (w0b_lo, w0b_hi)
                return lo[:, k, :] if k < KLO else hi[:, k - KLO, :]

            # HAM warmup: dummy matmuls with no data deps keep the PE busy
            # while the prologue DMAs stream. (A 12-MM batch arms the clock
            # gate outright, but the ~3.4 us prologue stalls re-throttle it
            # run-dependently, so the longer batch measured net-worse.)
            # Bank ps3_1 is first really used at o=3's B job, long after
            # the warmup retires.
            warm = constp.tile([P, 384], bf16)
            nc.gpsimd.memset(warm[:], 0)
            ps_warm = psump.tile([P, 384], f32, tag="ps3_1", name="warmps")
            NWARM = 4
            for i in range(NWARM):
                nc.tensor.matmul(
                    ps_warm[:],
                    warm[:, :P],
                    warm[:],
                    start=(i == 0),
                    stop=(i == NWARM - 1),
                )

            JOBS = ((0, S0, 0), (1, S1, S0))

            def evac(o, j, S, t0, ps):
                ot = outp.tile([P, S], bf16, tag=f"ot{j}", name=f"ot{o}_{j}")
                nc.vector.tensor_scalar(
                    ot[:],
                    ps[:],
                    bias_sb[:, o : o + 1],
                    None,
                    mybir.AluOpType.add,
                )
                rows = yt[o * P : (o + 1) * P]
                if o == ob - 1 and j == 1:
                    # Final store: halves on both queues so the two HBM
                    # write receipts (the dominant tail cost) overlap.
                    h = S // 2
                    nc.scalar.dma_start(rows[:, t0 : t0 + h], ot[:, :h])
                    nc.sync.dma_start(rows[:, t0 + h : t0 + S], ot[:, h:])
                else:
                    # The last o-blocks' stores ride the (by then idle) sync
                    # queue so they aren't stuck behind earlier stores.
                    eng = nc.sync if o >= ob - 2 else nc.scalar
                    eng.dma_start(rows[:, t0 : t0 + S], ot[:])

            # o=0: interleave the two jobs' k-loops so the PE consumes X
            # k-slabs at arrival pace while the rest of X streams in.
            ps0 = {
                j: psump.tile([P, S], f32, tag=f"ps0_{j}", name=f"ps0_{j}")
                for j, S, _ in JOBS
            }
            for k in range(kb):
                for j, S, t0 in JOBS:
                    nc.tensor.matmul(
                        ps0[j][:],
                        w0k(j, k),
                        xk(k, t0, S),
                        start=(k == 0),
                        stop=(k == kb - 1),
                    )
            for j, S, t0 in JOBS:
                evac(0, j, S, t0, ps0[j])

            # Filler matmuls in the known o=1 weight-wait stall: the early
            # phase is DMA-gated with ~50% PE duty, which never sustains the
            # ~3.4 us of activity HAM needs to unthrottle — so the steady
            # stream otherwise starts at half clock. These keep the PE busy
            # through the stall and arm HAM before the stream begins.
            for i in range(6):
                nc.tensor.matmul(
                    ps_warm[:],
                    warm[:, :P],
                    warm[:],
                    start=(i == 0),
                    stop=(i == 5),
                )

            for o in range(1, ob):
                wo = {}
                if o > 1:
                    for j, _, _ in JOBS:
                        wo[j] = wp.tile(
                            [P, kb, P], bf16, tag="w", name=f"w{o}_{j}"
                        )
                        eng = nc.sync if j == 0 else nc.scalar
                        eng.dma_start(wo[j][:], w[o][:, j])
                for j, S, t0 in JOBS:
                    ps = psump.tile(
                        [P, S], f32, tag=f"ps{o % 4}_{j}", name=f"ps{o}_{j}"
                    )
                    for k in range(kb):
                        nc.tensor.matmul(
                            ps[:],
                            w1k(j, k) if o == 1 else wo[j][:, k, :],
                            xk(k, t0, S),
                            start=(k == 0),
                            stop=(k == kb - 1),
                        )
                    evac(o, j, S, t0, ps)
    nc.finalize()
    return nc


def _prepare(inputs, weight, bias, group_sizes):
    """Build (or reuse) the program and the per-core input maps."""
    x = np.ascontiguousarray(np.asarray(inputs, dtype=np.float32))
    wt = np.asarray(weight, dtype=np.float32)
    b = np.asarray(bias, dtype=np.float32)
    g = np.asarray(group_sizes).astype(np.int64)

    t_tokens, n_in = x.shape
    n_exp, _, n_out = wt.shape
    assert n_exp == N_CORES, f"expected {N_CORES} experts, got {n_exp}"
    offs = np.concatenate([[0], np.cumsum(g)])
    assert offs[-1] == t_tokens, "group_sizes must sum to token count"

    S0, S1, assign = _pack([int(v) for v in g])
    slots = _slots_from_assign([int(v) for v in g], S0, S1, assign)
    TC = S0 + S1

    key = (S0, S1, n_in, n_out)
    if key not in _BUILD_CACHE:
        _BUILD_CACHE[key] = _build_program(S0, S1, n_in, n_out)
    nc = _BUILD_CACHE[key]

    kb, ob = n_in // P, n_out // P
    bf = ml_dtypes.bfloat16
    xb = x.astype(bf)                       # [T, n_in]
    wb = wt.astype(bf)                      # [E, n_in, n_out]
    bias_host = np.ascontiguousarray(b.reshape(ob, P).T.astype(np.float32))

    wtile: dict = {}

    def expert_w(e):
        # [ob, P(k within block), kb, P(o)]
        if e not in wtile:
            wtile[e] = np.ascontiguousarray(
                wb[e].reshape(kb, P, ob, P).transpose(2, 1, 0, 3)
            )
        return wtile[e]

    in_maps = []
    for c in range(N_CORES):
        xt_c = np.zeros((P, kb, TC), bf)
        w_c = np.zeros((ob, P, 2, kb, P), bf)
        for j, (slot, t0) in enumerate(((slots[c], 0), (slots[NSLOT + c], S0))):
            e, st, ln = slot
            if ln > 0:
                tok = xb[offs[e] + st : offs[e] + st + ln]  # [ln, n_in]
                xt_c[:, :, t0 : t0 + ln] = (
                    tok.T.reshape(kb, P, ln).transpose(1, 0, 2)
                )
            w_c[:, :, j] = expert_w(e)
        in_maps.append(
            {"xt": xt_c, "w": w_c, "bias": bias_host}
        )
    return nc, in_maps, (slots, S0, S1, offs, t_tokens, n_out)


def kernel(inputs, weight, bias, group_sizes):
    nc, in_maps, (slots, S0, S1, offs, t_tokens, n_out) = _prepare(
        inputs, weight, bias, group_sizes
    )
    res = run_bass_kernel_spmd(nc, in_maps, core_ids=list(range(N_CORES)))

    out = np.empty((t_tokens, n_out), np.float32)
    for c in range(N_CORES):
        ytc = res.results[c]["yt"]  # [n_out, TC] bf16
        for slot, t0 in ((slots[c], 0), (slots[NSLOT + c], S0)):
            e, st, ln = slot
            if ln > 0:
                out[offs[e] + st : offs[e] + st + ln] = (
                    ytc[:, t0 : t0 + ln].T.astype(np.float32)
                )
    return out


# revision 16
# speedup vs baseline: 1.0111x; 1.0111x over previous
"""MoE grouped linear (gmm) kernel for 8 Trainium2 NeuronCores.

Strategy (load-balanced expert parallel):
  - Tokens arrive pre-sorted by expert. Instead of one expert per core
    (which pads every core to the largest group), each core processes two
    fixed-size token slots (S0, S1): a bin-packing DP assigns each expert
    a_e slots of size S0 and b_e of size S1 (8 of each across the chip)
    so that a_e*S0 + b_e*S1 >= g_e. For the reference group sizes this
    yields (320, 224) = 544 tokens/core vs 768 for plain expert parallel.
  - All tensors are converted to bf16 host-side (round-to-nearest), which
    halves HBM traffic vs shipping fp32 and truncating on-chip; the output
    is returned as bf16 and upcast host-side (tolerance is 2e-2, measured
    error ~4e-3).
  - Per core: y^T[o-block] += W[job][o-block,k]^T @ X^T[k, slot] over 16
    k-blocks, accumulating in PSUM (one bank per (o%4, job)); evacuation
    fuses the bias add on the vector engine and writes bf16.
  - DMA: W slabs stream on the sync queue (per-o fused job pairs, 8 KiB
    per partition); X streams on the scalar queue so the two queues issue
    descriptors in parallel (each DMA_DIRECT2D costs ~0.7 us of issuing
    engine time). The o=0 job-A weights and the first X quarter are split
    fine so the first real matmul starts as early as possible after the
    ~7.6 us runtime init; HAM warmup matmuls bridge the wait.
Host then scatters per-slot outputs back to [T, Out] fp32.
"""

import numpy as np
import ml_dtypes

import concourse.bass as bass
from concourse import bacc
import concourse.mybir as mybir
import concourse.tile as tile
from concourse.bass_utils import run_bass_kernel_spmd

N_CORES = 8
P = 128
NSLOT = 8  # slots per size class (one per core)

_BUILD_CACHE: dict = {}


def _pack(g: list[int]):
    """Choose slot sizes (S0 >= S1) and per-expert slot counts.

    Returns (S0, S1, assign) with assign[e] = (a_e, b_e): expert e's tokens
    are covered by a_e slots of S0 tokens plus b_e slots of S1 tokens,
    sum(a) <= 8, sum(b) <= 8. Minimizes S0+S1 (per-core PE work), then the
    LDWEIGHTS-aware stream time max(S0,256)+max(S1,256).
    """
    E = len(g)
    total = sum(g)
    tc_min = max(64, -(-total // N_CORES // 32) * 32)

    def feasible(S0, S1):
        states = {(0, 0): None}
        hist = []
        for e in range(E):
            opts = []
            for a in range(NSLOT + 1):
                need = g[e] - a * S0
                b = max(0, -(-need // S1))
                if b <= NSLOT:
                    opts.append((a, b))
            new = {}
            for (A, B) in states:
                for (a, b) in opts:
                    if A + a <= NSLOT and B + b <= NSLOT and (A + a, B + b) not in new:
                        new[(A + a, B + b)] = (A, B, a, b)
            hist.append(new)
            states = new
            if not states:
                return None
        key = next(iter(states))
        assign = [None] * E
        for e in range(E - 1, -1, -1):
            A, B, a, b = hist[e][key]
            assign[e] = (a, b)
            key = (A, B)
        return assign

    for TC in range(tc_min, 1025, 32):
        cands = []
        for S1 in range(32, TC // 2 + 1, 32):
            S0 = TC - S1
            if S0 > 512:
                continue
            cands.append((max(S0, 256) + max(S1, 256), -S1, S0, S1))
        for _, _, S0, S1 in sorted(cands):
            asg = feasible(S0, S1)
            if asg is not None:
                return S0, S1, asg
    raise ValueError(f"no 2-slot packing found for group sizes {g}")


def _slots_from_assign(g, S0, S1, assign):
    """Materialize assign into 16 slots: [(expert, start_within_group, len)].
    Slots 0..7 have capacity S0 (one per core), 8..15 capacity S1."""
    s0_slots, s1_slots = [], []
    for e, (a, b) in enumerate(assign):
        rem, pos = g[e], 0
        for _ in range(a):
            take = min(S0, rem)
            s0_slots.append((e, pos, take))
            pos += take
            rem -= take
        for _ in range(b):
            take = min(S1, rem)
            s1_slots.append((e, pos, take))
            pos += take
            rem -= take
        assert rem == 0
    s0_slots += [(0, 0, 0)] * (NSLOT - len(s0_slots))
    s1_slots += [(0, 0, 0)] * (NSLOT - len(s1_slots))
    return s0_slots + s1_slots


def _build_program(S0: int, S1: int, n_in: int, n_out: int):
    kb = n_in // P   # contraction blocks
    ob = n_out // P  # output-row blocks
    TC = S0 + S1
    f32 = mybir.dt.float32
    bf16 = mybir.dt.bfloat16

    nc = bacc.Bacc(
        "TRN2", target_bir_lowering=False, debug=False, num_devices=N_CORES
    )
    # X^T resident in SBUF: [P(partition of k-block), kb, TC] bf16.
    xt = nc.dram_tensor("xt", [P, kb, TC], bf16, kind="ExternalInput")
    # W pre-tiled host-side: [ob, P(k within block), job, kb, P(o)] so one
    # fused per-o DMA moves both jobs' slabs as 8 KiB/partition segments.
    w = nc.dram_tensor("w", [ob, P, 2, kb, P], bf16, kind="ExternalInput")
    bias = nc.dram_tensor("bias", [P, ob], f32, kind="ExternalInput")
    yt = nc.dram_tensor("yt", [n_out, TC], bf16, kind="ExternalOutput")

    KLO = 4          # o=0 weight split: k < KLO arrives first

    with tile.TileContext(nc) as tc:
        with (
            tc.tile_pool(name="const", bufs=1) as constp,
            tc.tile_pool(name="xsb", bufs=1) as xp,
            tc.tile_pool(name="w0sb", bufs=1) as w0p,
            tc.tile_pool(name="wsb", bufs=8) as wp,
            tc.tile_pool(name="osb", bufs=4) as outp,
            tc.tile_pool(name="psum", bufs=1, space="PSUM") as psump,
        ):
            bias_sb = constp.tile([P, ob], f32)
            nc.scalar.dma_start(bias_sb[:], bias[:])

            def load_x(tag, k0, nk):
                t = xp.tile([P, nk, TC], bf16, tag=tag, name=tag)
                nc.sync.dma_start(t[:], xt[:, k0 : k0 + nk, :])
                return t

            def load_w0(j, k0, k1, tag):
                t = w0p.tile([P, k1 - k0, P], bf16, tag=tag, name=tag)
                nc.sync.dma_start(t[:], w[0][:, j, k0:k1, :])
                return t

            # Single load queue (sync), explicit priority order: the o=0
            # weight heads and first X k-slabs go first so the first real
            # matmul can issue right after the ~7 us runtime init; the o=0
            # weight tails, remaining X, then per-(o,job) W slabs follow.
            # One queue keeps the full HBM rate on exactly the bytes the PE
            # needs next (two queues split bandwidth round-robin).
            w0a_lo = load_w0(0, 0, KLO, "w0alo")
            w0b_lo = load_w0(1, 0, KLO, "w0blo")
            xq0a = load_x("x0a", 0, 2)
            xq0b = load_x("x0b", 2, 2)
            w0a_hi = load_w0(0, KLO, kb, "w0ahi")
            w0b_hi = load_w0(1, KLO, kb, "w0bhi")
            xqs = [load_x(f"x{q}", 4 * q, 4) for q in range(1, 4)]

            # o=1's weights also arrive as lo/hi pieces: the single ~3.5 us
            # wait for a full w1 slab after the X tail re-throttles HAM (any
            # PE-idle gap > ~3.4 us drops the clock to 1.2 GHz); several
            # sub-window waits keep the PE clock at 2.4 GHz into the steady
            # stream.
            def load_w1(j, k0, k1, tag):
                t = w0p.tile([P, k1 - k0, P], bf16, tag=tag, name=tag)
                nc.sync.dma_start(t[:], w[1][:, j, k0:k1, :])
                return t

            w1 = {
                (0, 0): load_w1(0, 0, KLO, "w1alo"),
                (0, 1): load_w1(0, KLO, kb, "w1ahi"),
                (1, 0): load_w1(1, 0, KLO, "w1blo"),
                (1, 1): load_w1(1, KLO, kb, "w1bhi"),
            }

            def w1k(j, k):
                if k < KLO:
                    return w1[(j, 0)][:, k, :]
                return w1[(j, 1)][:, k - KLO, :]

            def xk(k, t0, S):
                if k < 2:
                    return xq0a[:, k, t0 : t0 + S]
                if k < 4:
                    return xq0b[:, k - 2, t0 : t0 + S]
                q = k // 4
                return xqs[q - 1][:, k % 4, t0 : t0 + S]

            def w0k(j, k):
                lo, hi = (w0a_lo, w0a_hi) if j == 0 else (w0b_lo, w0b_hi)
                return lo[:, k, :] if k < KLO else hi[:, k - KLO, :]

            # HAM warmup: dummy matmuls with no data deps keep the PE busy
            # while the prologue DMAs stream. (A 12-MM batch arms the clock
            # gate outright, but the ~3.4 us prologue stalls re-throttle it
            # run-dependently, so the longer batch measured net-worse.)
            # Bank ps3_1 is first really used at o=3's B job, long after
            # the warmup retires.
            warm = constp.tile([P, 384], bf16)
            nc.gpsimd.memset(warm[:], 0)
            ps_warm = psump.tile([P, 384], f32, tag="ps3_1", name="warmps")
            NWARM = 4
            for i in range(NWARM):
                nc.tensor.matmul(
                    ps_warm[:],
                    warm[:, :P],
                    warm[:],
                    start=(i == 0),
                    stop=(i == NWARM - 1),
                )

            JOBS = ((0, S0, 0), (1, S1, S0))

            def evac(o, j, S, t0, ps):
                ot = outp.tile([P, S], bf16, tag=f"ot{j}", name=f"ot{o}_{j}")
                nc.vector.tensor_scalar(
                    ot[:],
                    ps[:],
                    bias_sb[:, o : o + 1],
                    None,
                    mybir.AluOpType.add,
                )
                rows = yt[o * P : (o + 1) * P]
                if o == ob - 1 and j == 1:
                    # Final store: halves on both queues so the two HBM
                    # write receipts (the dominant tail cost) overlap.
                    h = S // 2
                    nc.scalar.dma_start(rows[:, t0 : t0 + h], ot[:, :h])
                    nc.sync.dma_start(rows[:, t0 + h : t0 + S], ot[:, h:])
                else:
                    # The last o-blocks' stores ride the (by then idle) sync
                    # queue so they aren't stuck behind earlier stores.
                    eng = nc.sync if o >= ob - 2 else nc.scalar
                    eng.dma_start(rows[:, t0 : t0 + S], ot[:])

            # o=0: interleave the two jobs' k-loops so the PE consumes X
            # k-slabs at arrival pace while the rest of X streams in.
            ps0 = {
                j: psump.tile([P, S], f32, tag=f"ps0_{j}", name=f"ps0_{j}")
                for j, S, _ in JOBS
            }
            for k in range(kb):
                for j, S, t0 in JOBS:
                    nc.tensor.matmul(
                        ps0[j][:],
                        w0k(j, k),
                        xk(k, t0, S),
                        start=(k == 0),
                        stop=(k == kb - 1),
                    )
            for j, S, t0 in JOBS:
                evac(0, j, S, t0, ps0[j])

            # Filler matmuls in the known o=1 weight-wait stall: the early
            # phase is DMA-gated with ~50% PE duty, which never sustains the
            # ~3.4 us of activity HAM needs to unthrottle — so the steady
            # stream otherwise starts at half clock. These keep the PE busy
            # through the stall and arm HAM before the stream begins.
            for i in range(6):
                nc.tensor.matmul(
                    ps_warm[:],
                    warm[:, :P],
                    warm[:],
                    start=(i == 0),
                    stop=(i == 5),
                )

            for o in range(1, ob):
                wo = {}
                if o > 1:
                    for j, _, _ in JOBS:
                        wo[j] = wp.tile(
                            [P, kb, P], bf16, tag="w", name=f"w{o}_{j}"
                        )
                        nc.sync.dma_start(wo[j][:], w[o][:, j])
                for j, S, t0 in JOBS:
                    ps = psump.tile(
                        [P, S], f32, tag=f"ps{o % 4}_{j}", name=f"ps{o}_{j}"
                    )
                    for k in range(kb):
                        nc.tensor.matmul(
                            ps[:],
                            w1k(j, k) if o == 1 else wo[j][:, k, :],
                            xk(k, t0, S),
                            start=(k == 0),
                            stop=(k == kb - 1),
                        )
                    evac(o, j, S, t0, ps)
    nc.finalize()
    return nc


def _prepare(inputs, weight, bias, group_sizes):
    """Build (or reuse) the program and the per-core input maps."""
    x = np.ascontiguousarray(np.asarray(inputs, dtype=np.float32))
    wt = np.asarray(weight, dtype=np.float32)
    b = np.asarray(bias, dtype=np.float32)
    g = np.asarray(group_sizes).astype(np.int64)

    t_tokens, n_in = x.shape
    n_exp, _, n_out = wt.shape
    assert n_exp == N_CORES, f"expected {N_CORES} experts, got {n_exp}"
    offs = np.concatenate([[0], np.cumsum(g)])
    assert offs[-1] == t_tokens, "group_sizes must sum to token count"

    S0, S1, assign = _pack([int(v) for v in g])
    slots = _slots_from_assign([int(v) for v in g], S0, S1, assign)
    TC = S0 + S1

    key = (S0, S1, n_in, n_out)
    if key not in _BUILD_CACHE:
        _BUILD_CACHE[key] = _build_program(S0, S1, n_in, n_out)
    nc = _BUILD_CACHE[key]

    kb, ob = n_in // P, n_out // P
    bf = ml_dtypes.bfloat16
    xb = x.astype(bf)                       # [T, n_in]
    wb = wt.astype(bf)                      # [E, n_in, n_out]
    bias_host = np.ascontiguousarray(b.reshape(ob, P).T.astype(np.float32))

    wtile: dict = {}

    def expert_w(e):
        # [ob, P(k within block), kb, P(o)]
        if e not in wtile:
            wtile[e] = np.ascontiguousarray(
                wb[e].reshape(kb, P, ob, P).transpose(2, 1, 0, 3)
            )
        return wtile[e]

    in_maps = []
    for c in range(N_CORES):
        xt_c = np.zeros((P, kb, TC), bf)
        w_c = np.zeros((ob, P, 2, kb, P), bf)
        for j, (slot, t0) in enumerate(((slots[c], 0), (slots[NSLOT + c], S0))):
            e, st, ln = slot
            if ln > 0:
                tok = xb[offs[e] + st : offs[e] + st + ln]  # [ln, n_in]
                xt_c[:, :, t0 : t0 + ln] = (
                    tok.T.reshape(kb, P, ln).transpose(1, 0, 2)
                )
            w_c[:, :, j] = expert_w(e)
        in_maps.append(
            {"xt": xt_c, "w": w_c, "bias": bias_host}
        )
    return nc, in_maps, (slots, S0, S1, offs, t_tokens, n_out)


def kernel(inputs, weight, bias, group_sizes):
    nc, in_maps, (slots, S0, S1, offs, t_tokens, n_out) = _prepare(
        inputs, weight, bias, group_sizes
    )
    res = run_bass_kernel_spmd(nc, in_maps, core_ids=list(range(N_CORES)))

    out = np.empty((t_tokens, n_out), np.float32)
    for c in range(N_CORES):
        ytc = res.results[c]["yt"]  # [n_out, TC] bf16
        for slot, t0 in ((slots[c], 0), (slots[NSLOT + c], S0)):
            e, st, ln = slot
            if ln > 0:
                out[offs[e] + st : offs[e] + st + ln] = (
                    ytc[:, t0 : t0 + ln].T.astype(np.float32)
                )
    return out
